# revision 1
# baseline (speedup 1.0000x reference)
"""Trainium2 kernel for nn_Decoder_52664888983802.

est = einsum('bckE,wE->bckw', mixture_w, basis_weight); out = overlap_add(est, 8).

Sharding: batch dim (8) -> one batch row per NeuronCore (data parallel, no
collectives). Each core: mix [2, 16000, 512] f32 -> out [2, 128008] f32.
Measured ~250-280 us on silicon (HBM roofline ~185 us; rel err ~2.4e-3).

Per-core pipeline, 512-frame strips, mix path in bf16:
  SWDGE DMA load with f32->bf16 cast, raw [128, 4, 512] (frames on partitions)
  -> 16 PE transposes via identity => mixT chunks [128 e, 512 f] in PSUM
  -> PSUM->SBUF copies (split DVE/ACT halves)
  -> 4 accumulating bf16 matmuls, stationary wt [128, 128] (W1 at cols 0-7,
     W2 at cols 32-39, rest zero; 128 cols keeps FWL on) => est [128, nf] PSUM
  -> copy est PSUM->SBUF (DVE/ACT halves) with 1-col halo
  -> overlap-add folded into the output transpose: per 128-col block, two
     accumulating K=128 is_transpose matmuls whose rhs are one-hot column
     selectors (F rows 0-7, S rows 32-39 shifted one col; K<128 matmuls
     fault at runtime on this stack, hence the selector trick)
     => ct [128, 8] PSUM -> SBUF -> strided DMA out (32B runs)
  Final subframe j=16000 is DMA'd straight from est_sb's S rows.
  The output side is traced one strip late (software pipelining), and
  _prune_redundant_waits post-processes Tile's semaphores: several hw
  instruction structs accept a single foreign sync wait, so transitively
  implied waits are dropped (sems are monotonic and dispatch is in-order)
  and serial-engine self-waits are removed when paired with a data wait.
"""

import math
import sys

sys.path.insert(0, "/opt/trn_rl_repo")

import numpy as np

import concourse.bass as bass
import concourse.mybir as mybir
import concourse.tile as tile
from concourse.bass_utils import run_bass_kernel_spmd

F32 = mybir.dt.float32
F32R = mybir.dt.float32r

B, C, F, E, W = 8, 2, 16000, 512, 16
HALF = W // 2
SOFF = 32  # partition offset of the S-half in est
OUTLEN = HALF * (F - 1) + W  # 128008
N_CORES = 8


def build_decoder(C=C, F=F, E=E, W=W, STRIP=512, mix_dt="bf16"):
    HALF = W // 2
    NCHUNK = E // 128
    OUTLEN = HALF * (F - 1) + W

    mdt = {"bf16": mybir.dt.bfloat16, "f32r": F32R, "f32": F32}[mix_dt]
    cast_dma = mix_dt == "bf16"
    nc = bass.Bass()
    mix = nc.declare_dram_parameter(
        "mixture_w", [C, F, E], F32 if cast_dma else mdt, isOutput=False
    )
    wt = nc.declare_dram_parameter("wt", [E, 128], mdt, isOutput=False)
    id128 = nc.declare_dram_parameter("id128", [128, 128], mdt, isOutput=False)
    sel = nc.declare_dram_parameter("sel", [128, W], F32, isOutput=False)
    out = nc.declare_dram_parameter("out", [C, OUTLEN], F32, isOutput=True)

    nstrips = math.ceil(F / STRIP)

    with tile.TileContext(nc) as tc:
        with (
            tc.tile_pool(name="consts", bufs=1) as consts,
            tc.tile_pool(name="rawp", bufs=5) as rawp,
            tc.tile_pool(name="mixtp", bufs=6) as mixtp,
            tc.tile_pool(name="estsbp", bufs=3) as estsbp,
            tc.tile_pool(name="ctsbp", bufs=3) as ctsbp,
            tc.tile_pool(name="ptransp", bufs=3, space="PSUM") as ptransp,
            tc.tile_pool(name="pestp", bufs=3, space="PSUM") as pestp,
            tc.tile_pool(name="pctp", bufs=2, space="PSUM") as pctp,
        ):
            id128_sb = consts.tile([128, 128], mdt)
            nc.sync.dma_start(out=id128_sb[:], in_=id128[:])
            # selector: cols 0-7 pick est rows 0-7 (F), cols 8-15 pick rows
            # 32-39 (S) -- K=128 transposes only (K<128 faults at runtime)
            sel_sb = consts.tile([128, W], F32)
            nc.sync.dma_start(out=sel_sb[:], in_=sel[:])
            wt_sb = consts.tile([128, NCHUNK, 128], mdt)
            nc.sync.dma_start(out=wt_sb[:], in_=wt.rearrange("(q p) w -> p q w", p=128))

            # Warm-up PE ops: consume each const right after its DMA so that
            # steady-state PE instructions never need more than one
            # cross-engine wait (the f32r self-loading LDWEIGHTS struct has a
            # single sync-wait slot).
            warm = ptransp.tile([128, 128], mdt, tag="ptr", name="warm_t")
            nc.tensor.transpose(warm[:], id128_sb[:], id128_sb[:])
            warm2 = pctp.tile([W, W], F32, tag="ct", name="warm_ct")
            nc.tensor.matmul(
                warm2[:], lhsT=sel_sb[:], rhs=sel_sb[:], is_transpose=True
            )
            warm3 = pestp.tile([128, HALF], F32, tag="est", name="warm_mm")
            nc.tensor.matmul(
                warm3[:], lhsT=wt_sb[:, 0, :], rhs=wt_sb[:, 0, :HALF]
            )


            prev_estsb = None

            def emit_tail(c, s, f0, nf, last, est):
                # Deferred output side of a strip: traced one strip late so
                # the scheduler interleaves the next strip's PE work with
                # these DVE/ACT copies (software pipelining).
                nonlocal prev_estsb
                # est_sb col 0 is the halo: previous strip's last frame
                estsb = estsbp.tile(
                    [128, STRIP + 2], F32, tag="estsb", name=f"estsb_{c}_{s}"
                )
                half_nf = nf // 2
                nc.vector.tensor_copy(
                    out=estsb[:, 1 : 1 + half_nf], in_=est[:, :half_nf]
                )
                nc.scalar.copy(
                    out=estsb[:, 1 + half_nf : 1 + nf], in_=est[:, half_nf:nf]
                )
                if s == 0:
                    nc.vector.memset(estsb[:, 0:1], 0.0)
                else:
                    nc.vector.tensor_copy(
                        out=estsb[:, 0:1], in_=prev_estsb[:, STRIP : STRIP + 1]
                    )
                prev_estsb = estsb

                # output transpose with overlap-add folded in (K=128,
                # selector picks F rows 0-7 / S rows 32-39). Each output row
                # holds a PAIR of subframes (2*j2, 2*j2+1) so the out-DMA
                # writes 64B runs instead of 32B:
                # ct[j2, 0:8]  = F[u0+2j2]   + S[u0+2j2-1]
                # ct[j2, 8:16] = F[u0+2j2+1] + S[u0+2j2]
                nblk = nf // 256
                np_rows = 128 if nblk > 0 else 0
                if nblk == 0:  # 128-frame last strip: one 64-row block
                    nblk, np_rows = 1, nf // 2
                ct = pctp.tile([128, nblk * W], F32, tag="ct", name=f"ct_{c}_{s}")
                for t in range(nblk):
                    w0 = t * 256
                    ev = estsb[:, 1 + w0 : 1 + w0 + 2 * np_rows].rearrange(
                        "p (j two) -> p j two", two=2
                    )[:, :, 0]
                    od = estsb[:, 2 + w0 : 2 + w0 + 2 * np_rows].rearrange(
                        "p (j two) -> p j two", two=2
                    )[:, :, 0]
                    sv = estsb[:, w0 : w0 + 2 * np_rows].rearrange(
                        "p (j two) -> p j two", two=2
                    )[:, :, 0]
                    nc.tensor.matmul(
                        ct[0:np_rows, t * W : t * W + HALF],
                        lhsT=ev, rhs=sel_sb[:, 0:HALF],
                        is_transpose=True, start=True, stop=False,
                    )
                    nc.tensor.matmul(
                        ct[0:np_rows, t * W : t * W + HALF],
                        lhsT=sv, rhs=sel_sb[:, HALF:W],
                        is_transpose=True, start=False, stop=True,
                    )
                    nc.tensor.matmul(
                        ct[0:np_rows, t * W + HALF : t * W + W],
                        lhsT=od, rhs=sel_sb[:, 0:HALF],
                        is_transpose=True, start=True, stop=False,
                    )
                    nc.tensor.matmul(
                        ct[0:np_rows, t * W + HALF : t * W + W],
                        lhsT=ev, rhs=sel_sb[:, HALF:W],
                        is_transpose=True, start=False, stop=True,
                    )

                ctsb = ctsbp.tile(
                    [128, nblk, W], F32, tag="ctsb", name=f"ctsb_{c}_{s}"
                )
                nc.vector.tensor_copy(
                    out=ctsb[0:np_rows],
                    in_=ct[0:np_rows].rearrange("p (t w) -> p t w", w=W),
                )
                with tc.high_priority(offset=-150):
                    nc.sync.dma_start(
                        out=out[
                            c, f0 * HALF : f0 * HALF + nblk * np_rows * W
                        ].rearrange("(t p w) -> p t w", p=np_rows, w=W),
                        in_=ctsb[0:np_rows],
                    )
                if last:
                    # final subframe j=F: S-half of the last frame,
                    # straight from estsb (no M<128 matmul)
                    nc.sync.dma_start(
                        out=out[c, F * HALF : F * HALF + HALF].rearrange(
                            "(p w) -> p w", p=HALF
                        ),
                        in_=estsb[SOFF : SOFF + HALF, nf : nf + 1],
                    )
                # absorb each out-DMA read-completion (WAR) into a DVE
                # write so the next strip's tile writers need no DMA wait
                nc.vector.memset(ctsb[0:1, 0:1, 0:1], 0.0)
                if last:
                    nc.vector.memset(estsb[SOFF : SOFF + 1, nf : nf + 1], 0.0)

            pending = None
            raw = None
            raw_eng = nc.gpsimd if cast_dma else nc.sync
            for c in range(C):
                for s in range(nstrips):
                    f0 = s * STRIP
                    nf = min(STRIP, F - f0)
                    last = s == nstrips - 1
                    assert nf % 128 == 0
                    tb = nf // 128

                    raw = rawp.tile([128, STRIP // 128, E], mdt, tag="raw", name=f"raw_{c}_{s}")
                    with tc.high_priority(offset=90):
                        raw_eng.dma_start(
                            out=raw[:, :tb, :],
                            in_=mix[c, f0 : f0 + nf, :].rearrange(
                                "(t p) e -> p t e", p=128
                            ),
                        )

                    est = pestp.tile([128, STRIP], F32, tag="est", name=f"est_{c}_{s}")
                    for q in range(NCHUNK):
                        ptr = ptransp.tile(
                            [128, STRIP], mdt, tag="ptr", name=f"ptr_{c}_{s}_{q}"
                        )
                        for t in range(tb):
                            nc.tensor.transpose(
                                ptr[:, t * 128 : (t + 1) * 128],
                                raw[:, t, q * 128 : (q + 1) * 128],
                                id128_sb[:],
                            )
                        mxt = mixtp.tile(
                            [128, STRIP], mdt, tag="mixT", name=f"mxt_{c}_{s}_{q}"
                        )
                        hn = nf // 2
                        nc.vector.tensor_copy(out=mxt[:, :hn], in_=ptr[:, :hn])
                        nc.scalar.copy(out=mxt[:, hn:nf], in_=ptr[:, hn:nf])
                        nc.tensor.matmul(
                            est[:, :nf],
                            lhsT=wt_sb[:, q, :],
                            rhs=mxt[:, :nf],
                            start=(q == 0),
                            stop=(q == NCHUNK - 1),
                        )

                    if pending is not None:
                        emit_tail(*pending)
                    pending = (c, s, f0, nf, last, est)
            emit_tail(*pending)
    _prune_redundant_waits(nc)
    return nc


def _prune_redundant_waits(nc):
    """Drop semaphore waits that are transitively guaranteed.

    Tile's add_semaphores is per-proc minimal but not transitively minimal,
    and several hardware instruction structs (the f32r self-loading
    LDWEIGHTS, HWDGE ring entries) have a single sync-wait slot, so extra
    waits fail walrus codegen ("Too many sync wait commands").

    Soundness: semaphores only increase during execution, and every
    dispatch unit (engine NX, HWDGE ring) executes wait-then-dispatch in
    program order. Hence (a) knowledge carried by the same proc's earlier
    instructions remains true, and (b) a wait (s >= v) is redundant if the
    producer instruction that raised s to v itself had knowledge implying
    it. Additionally, PE-self waits on Matmults are WAW guards for the
    64-deep LDWEIGHTS reorder window; actual MATMULs are strict-FIFO
    (pc-monotone start and end) and LDWEIGHTS only reads SBUF whose
    writers' waits are kept, so they are droppable when another wait
    remains."""
    insts = [i for blk in nc.m.functions[0].blocks for i in blk.instructions]

    # Monotonicity only holds for sems that are never decremented. Engine and
    # DMA sems only see sem-inc / positive sem-add-imm; the barrier_* sems
    # (preamble + kernel tail) use sem-dec/sem-sub and are left untouched.
    unsafe_sems = set()
    for inst in insts:
        si = inst.sync_info
        if si is None:
            continue
        for u in si.on_update or []:
            if u.sync_type != "semaphore":
                continue
            if u.update_mode not in ("sem-inc", "sem-add-imm") or (
                u.update_mode == "sem-add-imm" and u.update_value <= 0
            ):
                unsafe_sems.add(u.id)

    R = {}  # proc -> {sem_id: guaranteed value}
    sem_cum = {}  # sem_id -> cumulative update value
    producer_know = {}  # sem_id -> [(cum_value, knowledge)] in order

    def implied(w, know):
        return know.get(w.id, 0) >= w.wait_value

    def know_of_wait(w):
        k = {w.id: w.wait_value}
        for cv, pk in producer_know.get(w.id, []):
            if cv >= w.wait_value:
                for s2, v2 in pk.items():
                    k[s2] = max(k.get(s2, 0), v2)
                break
        return k

    for inst in insts:
        si = inst.sync_info
        if si is None:
            continue
        waits = list(si.on_wait or [])
        p = str(inst.engine)
        base = dict(R.get(p, {}))
        if any(
            w.sync_type != "semaphore"
            or w.wait_reg is not None
            or w.wait_mode != "sem-ge-imm"
            or w.id in unsafe_sems
            for w in waits
        ):
            kept = waits  # don't touch register/non-sem/barrier waits
        else:
            kept = []
            live = [w for w in waits if not implied(w, base)]
            # prefer a single wait whose producer knowledge implies the rest
            single = None
            for w in live:
                kw = dict(base)
                for s2, v2 in know_of_wait(w).items():
                    kw[s2] = max(kw.get(s2, 0), v2)
                if all(o is w or implied(o, kw) for o in live):
                    single = w
                    break
            if single is not None:
                kept = [single]
            else:
                # greedy: keep a wait only if not implied by base + kept so far
                for w in sorted(live, key=lambda w: -w.wait_value):
                    if not implied(w, base):
                        kept.append(w)
                        for s2, v2 in know_of_wait(w).items():
                            base[s2] = max(base.get(s2, 0), v2)
            if len(kept) > 1:
                # serial in-order engines: own-sem waits are satisfied by
                # the time the instruction executes (PE MATMULs are
                # pc-monotone; DVE/ACT are single-pipeline serial)
                own = {"PE": "PE_", "DVE": "DVE_", "Activation": "Activation_"}.get(
                    str(inst.engine).split(".")[-1]
                )
                if own is not None:
                    nonself = [w for w in kept if not w.ant_name.startswith(own)]
                    if nonself:
                        kept = nonself
            if len(kept) != len(waits):
                si.on_wait = kept
        # final knowledge for this inst (all original waits still held at
        # runtime even if pruned from the emitted instruction)
        know = dict(R.get(p, {}))
        for w in waits:
            if (
                w.sync_type == "semaphore"
                and w.wait_reg is None
                and w.wait_mode == "sem-ge-imm"
                and w.id not in unsafe_sems
            ):
                for s2, v2 in know_of_wait(w).items():
                    know[s2] = max(know.get(s2, 0), v2)
        R[p] = know
        for u in si.on_update or []:
            if u.sync_type != "semaphore" or u.id in unsafe_sems:
                continue
            sem_cum[u.id] = sem_cum.get(u.id, 0) + u.update_value
            producer_know.setdefault(u.id, []).append((sem_cum[u.id], dict(know)))


_NC_CACHE = {}


def _get_nc(**kw):
    key = tuple(sorted(kw.items()))
    if key not in _NC_CACHE:
        _NC_CACHE[key] = build_decoder(**kw)
    return _NC_CACHE[key]


def prep_aux_inputs(basis_weight, mix_dt="bf16"):
    import ml_dtypes

    aux_np = {"bf16": ml_dtypes.bfloat16, "f32r": np.float32, "f32": np.float32}[
        mix_dt
    ]
    wt = np.zeros((E, 128), dtype=np.float32)
    wt[:, 0:HALF] = basis_weight.T[:, 0:HALF]
    wt[:, SOFF : SOFF + HALF] = basis_weight.T[:, HALF:W]
    id128 = np.eye(128, dtype=np.float32)
    sel = np.zeros((128, W), dtype=np.float32)
    for j in range(HALF):
        sel[j, j] = 1.0
        sel[SOFF + j, HALF + j] = 1.0
    return wt.astype(aux_np), id128.astype(aux_np), sel


def kernel(mixture_w, basis_weight, _trace=False, **build_kw):
    mixture_w = np.ascontiguousarray(mixture_w, dtype=np.float32)
    basis_weight = np.ascontiguousarray(basis_weight, dtype=np.float32)
    assert mixture_w.shape == (B, C, F, E), mixture_w.shape
    assert basis_weight.shape == (W, E), basis_weight.shape

    nc = _get_nc(**build_kw)
    wt, id128, sel = prep_aux_inputs(
        basis_weight, mix_dt=build_kw.get("mix_dt", "bf16")
    )
    in_maps = [
        {"mixture_w": mixture_w[i], "wt": wt, "id128": id128, "sel": sel}
        for i in range(N_CORES)
    ]
    res = run_bass_kernel_spmd(
        nc, in_maps, core_ids=list(range(N_CORES)), trace=_trace
    )
    out = np.stack([res.results[i]["out"] for i in range(N_CORES)], axis=0)
    if _trace:
        kernel.last_exec_time_ns = res.exec_time_ns
        kernel.last_result = res
    return out



# revision 5
# speedup vs baseline: 1.0047x; 1.0047x over previous
"""Trainium2 kernel for nn_Decoder_52664888983802.

est = einsum('bckE,wE->bckw', mixture_w, basis_weight); out = overlap_add(est, 8).

Sharding: batch dim (8) -> one batch row per NeuronCore (data parallel, no
collectives). Each core: mix [2, 16000, 512] f32 -> out [2, 128008] f32.
Measured ~250-280 us on silicon (HBM roofline ~185 us; rel err ~2.4e-3).

Per-core pipeline, 512-frame strips, mix path in bf16:
  SWDGE DMA load with f32->bf16 cast, raw [128, tb=4, 512] in P-MAJOR frame
  order: partition p holds frames f0+tb*p .. f0+tb*p+tb-1, i.e. one
  contiguous 8 KB HBM run per partition (128 big descriptors per strip
  instead of 516 2KB ones -- keeps Q7 SWDGE emission off the critical path
  and SDMA packets large).
  -> 16 PE transposes via identity => mixT chunks [128 e, 512 f] in PSUM,
     est column c = t*128+j <-> frame tb*j + t (block t = frames == t mod tb)
  -> PSUM->SBUF copies (split DVE/ACT halves)
  -> 4 accumulating bf16 matmuls, stationary wt [128, 128] (W1 at cols 0-7,
     W2 at cols 32-39, rest zero; 128 cols keeps FWL on) => est [128, nf] PSUM
  -> est PSUM->SBUF reordered as [halo | b_{tb-1} | b_0 .. b_{tb-2}], so the
     overlap-add shift-by-one-frame becomes contiguous 128-col slices:
     F slice of residue r = block r, S slice = block r-1 (r=0: halo+b_{tb-1}).
     estsb col 128 is always the strip's last frame (next strip's halo).
  -> overlap-add folded into the output transpose: per residue r, two
     accumulating K=128 is_transpose matmuls with one-hot column selectors
     (F rows 0-7, S rows 32-39; K<128 matmuls fault at runtime on this
     stack, hence the selector trick) => ct[j, r*8+k] = out subframe
     tb*j+r -> ct [128, tb*8] PSUM -> SBUF -> DMA out with one contiguous
     128B run per partition.
  Final subframe j=16000 is DMA'd straight from est_sb's S rows.
  The output side is traced one strip late (software pipelining), and
  _prune_redundant_waits post-processes Tile's semaphores: several hw
  instruction structs accept a single foreign sync wait, so transitively
  implied waits are dropped (sems are monotonic and dispatch is in-order)
  and serial-engine self-waits are removed when paired with a data wait.
"""

import math
import sys

sys.path.insert(0, "/opt/trn_rl_repo")

import numpy as np

import concourse.bass as bass
import concourse.mybir as mybir
import concourse.tile as tile
from concourse.bass_utils import run_bass_kernel_spmd

F32 = mybir.dt.float32
F32R = mybir.dt.float32r

B, C, F, E, W = 8, 2, 16000, 512, 16
HALF = W // 2
SOFF = 32  # partition offset of the S-half in est
OUTLEN = HALF * (F - 1) + W  # 128008
N_CORES = 8


def build_decoder(C=C, F=F, E=E, W=W, STRIP=512, mix_dt="bf16"):
    HALF = W // 2
    NCHUNK = E // 128
    OUTLEN = HALF * (F - 1) + W

    mdt = {"bf16": mybir.dt.bfloat16, "f32r": F32R, "f32": F32}[mix_dt]
    cast_dma = mix_dt == "bf16"
    nc = bass.Bass()
    mix = nc.declare_dram_parameter(
        "mixture_w", [C, F, E], F32 if cast_dma else mdt, isOutput=False
    )
    wt = nc.declare_dram_parameter("wt", [E, 128], mdt, isOutput=False)
    id128 = nc.declare_dram_parameter("id128", [128, 128], mdt, isOutput=False)
    sel = nc.declare_dram_parameter("sel", [128, W], F32, isOutput=False)
    out = nc.declare_dram_parameter("out", [C, OUTLEN], F32, isOutput=True)

    nstrips = math.ceil(F / STRIP)

    with tile.TileContext(nc) as tc:
        with (
            tc.tile_pool(name="consts", bufs=1) as consts,
            tc.tile_pool(name="rawp", bufs=8) as rawp,
            tc.tile_pool(name="mixtp", bufs=6) as mixtp,
            tc.tile_pool(name="estsbp", bufs=3) as estsbp,
            tc.tile_pool(name="ctsbp", bufs=3) as ctsbp,
            tc.tile_pool(name="ptransp", bufs=3, space="PSUM") as ptransp,
            tc.tile_pool(name="pestp", bufs=3, space="PSUM") as pestp,
            tc.tile_pool(name="pctp", bufs=2, space="PSUM") as pctp,
        ):
            id128_sb = consts.tile([128, 128], mdt)
            nc.sync.dma_start(out=id128_sb[:], in_=id128[:])
            # selector: cols 0-7 pick est rows 0-7 (F), cols 8-15 pick rows
            # 32-39 (S) -- K=128 transposes only (K<128 faults at runtime)
            sel_sb = consts.tile([128, W], F32)
            nc.sync.dma_start(out=sel_sb[:], in_=sel[:])
            wt_sb = consts.tile([128, NCHUNK, 128], mdt)
            nc.sync.dma_start(out=wt_sb[:], in_=wt.rearrange("(q p) w -> p q w", p=128))

            # Warm-up PE ops: consume each const right after its DMA so that
            # steady-state PE instructions never need more than one
            # cross-engine wait (the f32r self-loading LDWEIGHTS struct has a
            # single sync-wait slot).
            warm = ptransp.tile([128, 128], mdt, tag="ptr", name="warm_t")
            nc.tensor.transpose(warm[:], id128_sb[:], id128_sb[:])
            warm2 = pctp.tile([W, W], F32, tag="ct", name="warm_ct")
            nc.tensor.matmul(
                warm2[:], lhsT=sel_sb[:], rhs=sel_sb[:], is_transpose=True
            )
            warm3 = pestp.tile([128, HALF], F32, tag="est", name="warm_mm")
            nc.tensor.matmul(
                warm3[:], lhsT=wt_sb[:, 0, :], rhs=wt_sb[:, 0, :HALF]
            )


            prev_estsb = None

            def emit_tail(c, s, f0, nf, last, est):
                # Deferred output side of a strip: traced one strip late so
                # the scheduler interleaves the next strip's PE work with
                # these DVE/ACT copies (software pipelining).
                nonlocal prev_estsb
                tb = nf // 128
                # est_sb col 0 is the halo (previous strip's last frame);
                # block order [halo | b_{tb-1} | b_0 .. b_{tb-2}] so every
                # F/S slice below is 128 contiguous cols.
                estsb = estsbp.tile(
                    [128, STRIP + 2], F32, tag="estsb", name=f"estsb_{c}_{s}"
                )
                # b_{tb-1} first (cols 1..129) -- estsb col 128 = last frame
                nc.vector.tensor_copy(
                    out=estsb[:, 1:129], in_=est[:, (tb - 1) * 128 : nf]
                )
                if tb > 1:
                    # remaining blocks b_0..b_{tb-2}, split DVE/ACT
                    hn = ((tb - 1) // 2) * 128
                    if hn:
                        nc.vector.tensor_copy(
                            out=estsb[:, 129 : 129 + hn], in_=est[:, 0:hn]
                        )
                    nc.scalar.copy(
                        out=estsb[:, 129 + hn : 129 + (tb - 1) * 128],
                        in_=est[:, hn : (tb - 1) * 128],
                    )
                if s == 0:
                    nc.vector.memset(estsb[:, 0:1], 0.0)
                else:
                    nc.vector.tensor_copy(
                        out=estsb[:, 0:1], in_=prev_estsb[:, 128:129]
                    )
                prev_estsb = estsb

                # output transpose with overlap-add folded in (K=128,
                # selector picks F rows 0-7 / S rows 32-39).
                # ct[j, r*8+k] = F[tb*j+r][k] + S[tb*j+r-1][k]
                def fstart(r):
                    return 1 if r == tb - 1 else 129 + 128 * r

                ct = pctp.tile([128, tb * HALF], F32, tag="ct", name=f"ct_{c}_{s}")
                for r in range(tb):
                    fs = fstart(r)
                    ss = 0 if r == 0 else fstart(r - 1)
                    nc.tensor.matmul(
                        ct[:, r * HALF : (r + 1) * HALF],
                        lhsT=estsb[:, fs : fs + 128], rhs=sel_sb[:, 0:HALF],
                        is_transpose=True, start=True, stop=False,
                    )
                    nc.tensor.matmul(
                        ct[:, r * HALF : (r + 1) * HALF],
                        lhsT=estsb[:, ss : ss + 128], rhs=sel_sb[:, HALF:W],
                        is_transpose=True, start=False, stop=True,
                    )

                ctsb = ctsbp.tile(
                    [128, tb * HALF], F32, tag="ctsb", name=f"ctsb_{c}_{s}"
                )
                nc.vector.tensor_copy(out=ctsb[:, : tb * HALF], in_=ct[:])
                with tc.high_priority(offset=-150):
                    # partition j covers out subframes tb*j+0 .. tb*j+tb-1:
                    # one contiguous tb*32-byte run per partition
                    nc.sync.dma_start(
                        out=out[
                            c, f0 * HALF : (f0 + nf) * HALF
                        ].rearrange("(p x) -> p x", p=128),
                        in_=ctsb[:, : tb * HALF],
                    )
                if last:
                    # final subframe j=F: S-half of the last frame,
                    # straight from estsb (no M<128 matmul)
                    nc.sync.dma_start(
                        out=out[c, F * HALF : F * HALF + HALF].rearrange(
                            "(p w) -> p w", p=HALF
                        ),
                        in_=estsb[SOFF : SOFF + HALF, 128:129],
                    )
                # absorb each out-DMA read-completion (WAR) into a DVE
                # write so the next strip's tile writers need no DMA wait
                nc.vector.memset(ctsb[0:1, 0:1], 0.0)
                if last:
                    nc.vector.memset(estsb[SOFF : SOFF + 1, 128:129], 0.0)

            pending = None
            raw = None
            raw_eng = nc.gpsimd if cast_dma else nc.sync
            for c in range(C):
                for s in range(nstrips):
                    f0 = s * STRIP
                    nf = min(STRIP, F - f0)
                    last = s == nstrips - 1
                    assert nf % 128 == 0
                    tb = nf // 128

                    raw = rawp.tile([128, STRIP // 128, E], mdt, tag="raw", name=f"raw_{c}_{s}")
                    with tc.high_priority(offset=90):
                        # p-major: partition p <- frames f0+tb*p..+tb-1, one
                        # contiguous tb*2KB HBM read run per partition
                        raw_eng.dma_start(
                            out=raw[:, :tb, :],
                            in_=mix[c, f0 : f0 + nf, :].rearrange(
                                "(p t) e -> p t e", p=128
                            ),
                        )

                    est = pestp.tile([128, STRIP], F32, tag="est", name=f"est_{c}_{s}")
                    for q in range(NCHUNK):
                        ptr = ptransp.tile(
                            [128, STRIP], mdt, tag="ptr", name=f"ptr_{c}_{s}_{q}"
                        )
                        for t in range(tb):
                            nc.tensor.transpose(
                                ptr[:, t * 128 : (t + 1) * 128],
                                raw[:, t, q * 128 : (q + 1) * 128],
                                id128_sb[:],
                            )
                        mxt = mixtp.tile(
                            [128, STRIP], mdt, tag="mixT", name=f"mxt_{c}_{s}_{q}"
                        )
                        hn = nf // 2
                        nc.vector.tensor_copy(out=mxt[:, :hn], in_=ptr[:, :hn])
                        nc.scalar.copy(out=mxt[:, hn:nf], in_=ptr[:, hn:nf])
                        nc.tensor.matmul(
                            est[:, :nf],
                            lhsT=wt_sb[:, q, :],
                            rhs=mxt[:, :nf],
                            start=(q == 0),
                            stop=(q == NCHUNK - 1),
                        )

                    if pending is not None:
                        emit_tail(*pending)
                    pending = (c, s, f0, nf, last, est)
            emit_tail(*pending)
    _prune_redundant_waits(nc)
    return nc


def _prune_redundant_waits(nc):
    """Drop semaphore waits that are transitively guaranteed.

    Tile's add_semaphores is per-proc minimal but not transitively minimal,
    and several hardware instruction structs (the f32r self-loading
    LDWEIGHTS, HWDGE ring entries) have a single sync-wait slot, so extra
    waits fail walrus codegen ("Too many sync wait commands").

    Soundness: semaphores only increase during execution, and every
    dispatch unit (engine NX, HWDGE ring) executes wait-then-dispatch in
    program order. Hence (a) knowledge carried by the same proc's earlier
    instructions remains true, and (b) a wait (s >= v) is redundant if the
    producer instruction that raised s to v itself had knowledge implying
    it. Additionally, PE-self waits on Matmults are WAW guards for the
    64-deep LDWEIGHTS reorder window; actual MATMULs are strict-FIFO
    (pc-monotone start and end) and LDWEIGHTS only reads SBUF whose
    writers' waits are kept, so they are droppable when another wait
    remains."""
    insts = [i for blk in nc.m.functions[0].blocks for i in blk.instructions]

    # Monotonicity only holds for sems that are never decremented. Engine and
    # DMA sems only see sem-inc / positive sem-add-imm; the barrier_* sems
    # (preamble + kernel tail) use sem-dec/sem-sub and are left untouched.
    unsafe_sems = set()
    for inst in insts:
        si = inst.sync_info
        if si is None:
            continue
        for u in si.on_update or []:
            if u.sync_type != "semaphore":
                continue
            if u.update_mode not in ("sem-inc", "sem-add-imm") or (
                u.update_mode == "sem-add-imm" and u.update_value <= 0
            ):
                unsafe_sems.add(u.id)

    R = {}  # proc -> {sem_id: guaranteed value}
    sem_cum = {}  # sem_id -> cumulative update value
    producer_know = {}  # sem_id -> [(cum_value, knowledge)] in order

    def implied(w, know):
        return know.get(w.id, 0) >= w.wait_value

    def know_of_wait(w):
        k = {w.id: w.wait_value}
        for cv, pk in producer_know.get(w.id, []):
            if cv >= w.wait_value:
                for s2, v2 in pk.items():
                    k[s2] = max(k.get(s2, 0), v2)
                break
        return k

    for inst in insts:
        si = inst.sync_info
        if si is None:
            continue
        waits = list(si.on_wait or [])
        p = str(inst.engine)
        base = dict(R.get(p, {}))
        if any(
            w.sync_type != "semaphore"
            or w.wait_reg is not None
            or w.wait_mode != "sem-ge-imm"
            or w.id in unsafe_sems
            for w in waits
        ):
            kept = waits  # don't touch register/non-sem/barrier waits
        else:
            kept = []
            live = [w for w in waits if not implied(w, base)]
            # prefer a single wait whose producer knowledge implies the rest
            single = None
            for w in live:
                kw = dict(base)
                for s2, v2 in know_of_wait(w).items():
                    kw[s2] = max(kw.get(s2, 0), v2)
                if all(o is w or implied(o, kw) for o in live):
                    single = w
                    break
            if single is not None:
                kept = [single]
            else:
                # greedy: keep a wait only if not implied by base + kept so far
                for w in sorted(live, key=lambda w: -w.wait_value):
                    if not implied(w, base):
                        kept.append(w)
                        for s2, v2 in know_of_wait(w).items():
                            base[s2] = max(base.get(s2, 0), v2)
            if len(kept) > 1:
                # serial in-order engines: own-sem waits are satisfied by
                # the time the instruction executes (PE MATMULs are
                # pc-monotone; DVE/ACT are single-pipeline serial)
                own = {"PE": "PE_", "DVE": "DVE_", "Activation": "Activation_"}.get(
                    str(inst.engine).split(".")[-1]
                )
                if own is not None:
                    nonself = [w for w in kept if not w.ant_name.startswith(own)]
                    if nonself:
                        kept = nonself
            if len(kept) != len(waits):
                si.on_wait = kept
        # final knowledge for this inst (all original waits still held at
        # runtime even if pruned from the emitted instruction)
        know = dict(R.get(p, {}))
        for w in waits:
            if (
                w.sync_type == "semaphore"
                and w.wait_reg is None
                and w.wait_mode == "sem-ge-imm"
                and w.id not in unsafe_sems
            ):
                for s2, v2 in know_of_wait(w).items():
                    know[s2] = max(know.get(s2, 0), v2)
        R[p] = know
        for u in si.on_update or []:
            if u.sync_type != "semaphore" or u.id in unsafe_sems:
                continue
            sem_cum[u.id] = sem_cum.get(u.id, 0) + u.update_value
            producer_know.setdefault(u.id, []).append((sem_cum[u.id], dict(know)))


_NC_CACHE = {}


def _get_nc(**kw):
    key = tuple(sorted(kw.items()))
    if key not in _NC_CACHE:
        _NC_CACHE[key] = build_decoder(**kw)
    return _NC_CACHE[key]


def prep_aux_inputs(basis_weight, mix_dt="bf16"):
    import ml_dtypes

    aux_np = {"bf16": ml_dtypes.bfloat16, "f32r": np.float32, "f32": np.float32}[
        mix_dt
    ]
    wt = np.zeros((E, 128), dtype=np.float32)
    wt[:, 0:HALF] = basis_weight.T[:, 0:HALF]
    wt[:, SOFF : SOFF + HALF] = basis_weight.T[:, HALF:W]
    id128 = np.eye(128, dtype=np.float32)
    sel = np.zeros((128, W), dtype=np.float32)
    for j in range(HALF):
        sel[j, j] = 1.0
        sel[SOFF + j, HALF + j] = 1.0
    return wt.astype(aux_np), id128.astype(aux_np), sel


def kernel(mixture_w, basis_weight, _trace=False, **build_kw):
    mixture_w = np.ascontiguousarray(mixture_w, dtype=np.float32)
    basis_weight = np.ascontiguousarray(basis_weight, dtype=np.float32)
    assert mixture_w.shape == (B, C, F, E), mixture_w.shape
    assert basis_weight.shape == (W, E), basis_weight.shape

    nc = _get_nc(**build_kw)
    wt, id128, sel = prep_aux_inputs(
        basis_weight, mix_dt=build_kw.get("mix_dt", "bf16")
    )
    in_maps = [
        {"mixture_w": mixture_w[i], "wt": wt, "id128": id128, "sel": sel}
        for i in range(N_CORES)
    ]
    res = run_bass_kernel_spmd(
        nc, in_maps, core_ids=list(range(N_CORES)), trace=_trace
    )
    out = np.stack([res.results[i]["out"] for i in range(N_CORES)], axis=0)
    if _trace:
        kernel.last_exec_time_ns = res.exec_time_ns
        kernel.last_result = res
    return out



# revision 9
# speedup vs baseline: 1.0441x; 1.0391x over previous
"""Trainium2 kernel for nn_Decoder_52664888983802.

est = einsum('bckE,wE->bckw', mixture_w, basis_weight); out = overlap_add(est, 8).

Sharding: batch dim (8) -> one batch row per NeuronCore (data parallel, no
collectives). Each core: mix [2, 16000, 512] f32 -> out [2, 128008] f32.
Measured ~250-280 us on silicon (HBM roofline ~185 us; rel err ~2.4e-3).

Per-core pipeline, 512-frame strips, mix path in bf16:
  SWDGE DMA load with f32->bf16 cast, raw [128, tb=4, 512] in P-MAJOR frame
  order: partition p holds frames f0+tb*p .. f0+tb*p+tb-1, i.e. one
  contiguous 8 KB HBM run per partition (128 big descriptors per strip
  instead of 516 2KB ones -- keeps Q7 SWDGE emission off the critical path
  and SDMA packets large).
  -> 16 PE transposes via identity => mixT chunks [128 e, 512 f] in PSUM,
     est column c = t*128+j <-> frame tb*j + t (block t = frames == t mod tb)
  -> PSUM->SBUF copies (split DVE/ACT halves)
  -> 4 accumulating bf16 matmuls, stationary wt [128, 128] (W1 at cols 0-7,
     W2 at cols 32-39, rest zero; 128 cols keeps FWL on) => est [128, nf] PSUM
  -> est PSUM->SBUF reordered as [halo | b_{tb-1} | b_0 .. b_{tb-2}], so the
     overlap-add shift-by-one-frame becomes contiguous 128-col slices:
     F slice of residue r = block r, S slice = block r-1 (r=0: halo+b_{tb-1}).
     estsb col 128 is always the strip's last frame (next strip's halo).
  -> overlap-add folded into the output transpose: per residue r, two
     accumulating K=128 is_transpose matmuls with one-hot column selectors
     (F rows 0-7, S rows 32-39; K<128 matmuls fault at runtime on this
     stack, hence the selector trick) => ct[j, r*8+k] = out subframe
     tb*j+r -> ct [128, tb*8] PSUM -> SBUF -> DMA out with one contiguous
     128B run per partition.
  Final subframe j=16000 is DMA'd straight from est_sb's S rows.
  The output side is traced one strip late (software pipelining), and
  _prune_redundant_waits post-processes Tile's semaphores: several hw
  instruction structs accept a single foreign sync wait, so transitively
  implied waits are dropped (sems are monotonic and dispatch is in-order)
  and serial-engine self-waits are removed when paired with a data wait.
"""

import math
import sys

sys.path.insert(0, "/opt/trn_rl_repo")

import numpy as np

import concourse.bass as bass
import concourse.mybir as mybir
import concourse.tile as tile
from concourse.bass_utils import run_bass_kernel_spmd

F32 = mybir.dt.float32
F32R = mybir.dt.float32r

B, C, F, E, W = 8, 2, 16000, 512, 16
HALF = W // 2
SOFF = 32  # partition offset of the S-half in est
OUTLEN = HALF * (F - 1) + W  # 128008
N_CORES = 8


def build_decoder(C=C, F=F, E=E, W=W, STRIP=512, mix_dt="bf16"):
    HALF = W // 2
    NCHUNK = E // 128
    OUTLEN = HALF * (F - 1) + W

    mdt = {"bf16": mybir.dt.bfloat16, "f32r": F32R, "f32": F32}[mix_dt]
    cast_dma = mix_dt == "bf16"
    nc = bass.Bass()
    mix = nc.declare_dram_parameter(
        "mixture_w", [C, F, E], F32 if cast_dma else mdt, isOutput=False
    )
    wt = nc.declare_dram_parameter("wt", [E, 128], mdt, isOutput=False)
    id128 = nc.declare_dram_parameter("id128", [128, 128], mdt, isOutput=False)
    sel = nc.declare_dram_parameter("sel", [128, W], F32, isOutput=False)
    out = nc.declare_dram_parameter("out", [C, OUTLEN], F32, isOutput=True)

    nstrips = math.ceil(F / STRIP)

    with tile.TileContext(nc) as tc:
        with (
            tc.tile_pool(name="consts", bufs=1) as consts,
            tc.tile_pool(name="rawp", bufs=8) as rawp,
            tc.tile_pool(name="mixtp", bufs=8) as mixtp,
            tc.tile_pool(name="estsbp", bufs=5) as estsbp,
            tc.tile_pool(name="ctsbp", bufs=4) as ctsbp,
            tc.tile_pool(name="ptransp", bufs=3, space="PSUM") as ptransp,
            tc.tile_pool(name="pestp", bufs=4, space="PSUM") as pestp,
            tc.tile_pool(name="pctp", bufs=1, space="PSUM") as pctp,
        ):
            id128_sb = consts.tile([128, 128], mdt)
            nc.sync.dma_start(out=id128_sb[:], in_=id128[:])
            # selector: cols 0-7 pick est rows 0-7 (F), cols 8-15 pick rows
            # 32-39 (S) -- K=128 transposes only (K<128 faults at runtime)
            sel_sb = consts.tile([128, W], F32)
            nc.sync.dma_start(out=sel_sb[:], in_=sel[:])
            wt_sb = consts.tile([128, NCHUNK, 128], mdt)
            nc.sync.dma_start(out=wt_sb[:], in_=wt.rearrange("(q p) w -> p q w", p=128))

            # Warm-up PE ops: consume each const right after its DMA so that
            # steady-state PE instructions never need more than one
            # cross-engine wait (the f32r self-loading LDWEIGHTS struct has a
            # single sync-wait slot).
            warm = ptransp.tile([128, 128], mdt, tag="ptr", name="warm_t")
            nc.tensor.transpose(warm[:], id128_sb[:], id128_sb[:])
            warm2 = pctp.tile([W, W], F32, tag="ct", name="warm_ct")
            nc.tensor.matmul(
                warm2[:], lhsT=sel_sb[:], rhs=sel_sb[:], is_transpose=True
            )
            warm3 = pestp.tile([128, HALF], F32, tag="est", name="warm_mm")
            nc.tensor.matmul(
                warm3[:], lhsT=wt_sb[:, 0, :], rhs=wt_sb[:, 0, :HALF]
            )


            prev_estsb = None

            def emit_tail(c, s, f0, nf, last, est):
                # Deferred output side of a strip: traced one strip late so
                # the scheduler interleaves the next strip's PE work with
                # these DVE/ACT copies (software pipelining).
                nonlocal prev_estsb
                tb = nf // 128
                # est_sb col 0 is the halo (previous strip's last frame);
                # block order [halo | b_{tb-1} | b_0 .. b_{tb-2}] so every
                # F/S slice below is 128 contiguous cols.
                estsb = estsbp.tile(
                    [128, STRIP + 2], F32, tag="estsb", name=f"estsb_{c}_{s}"
                )
                # b_{tb-1} first (cols 1..129) -- estsb col 128 = last frame
                nc.vector.tensor_copy(
                    out=estsb[:, 1:129], in_=est[:, (tb - 1) * 128 : nf]
                )
                if tb > 1:
                    # remaining blocks b_0..b_{tb-2}, split DVE/ACT
                    hn = ((tb - 1) // 2) * 128
                    if hn:
                        nc.vector.tensor_copy(
                            out=estsb[:, 129 : 129 + hn], in_=est[:, 0:hn]
                        )
                    nc.scalar.copy(
                        out=estsb[:, 129 + hn : 129 + (tb - 1) * 128],
                        in_=est[:, hn : (tb - 1) * 128],
                    )
                if s == 0:
                    nc.vector.memset(estsb[:, 0:1], 0.0)
                else:
                    nc.vector.tensor_copy(
                        out=estsb[:, 0:1], in_=prev_estsb[:, 128:129]
                    )
                prev_estsb = estsb

                # output transpose with overlap-add folded in (K=128,
                # selector picks F rows 0-7 / S rows 32-39).
                # ct[j, r*8+k] = F[tb*j+r][k] + S[tb*j+r-1][k]
                def fstart(r):
                    return 1 if r == tb - 1 else 129 + 128 * r

                ct = pctp.tile([128, tb * HALF], F32, tag="ct", name=f"ct_{c}_{s}")
                for r in range(tb):
                    fs = fstart(r)
                    ss = 0 if r == 0 else fstart(r - 1)
                    nc.tensor.matmul(
                        ct[:, r * HALF : (r + 1) * HALF],
                        lhsT=estsb[:, fs : fs + 128], rhs=sel_sb[:, 0:HALF],
                        is_transpose=True, start=True, stop=False,
                    )
                    nc.tensor.matmul(
                        ct[:, r * HALF : (r + 1) * HALF],
                        lhsT=estsb[:, ss : ss + 128], rhs=sel_sb[:, HALF:W],
                        is_transpose=True, start=False, stop=True,
                    )

                ctsb = ctsbp.tile(
                    [128, tb * HALF], F32, tag="ctsb", name=f"ctsb_{c}_{s}"
                )
                nc.vector.tensor_copy(out=ctsb[:, : tb * HALF], in_=ct[:])
                with tc.high_priority(offset=-150):
                    # partition j covers out subframes tb*j+0 .. tb*j+tb-1:
                    # one contiguous tb*32-byte run per partition
                    nc.sync.dma_start(
                        out=out[
                            c, f0 * HALF : (f0 + nf) * HALF
                        ].rearrange("(p x) -> p x", p=128),
                        in_=ctsb[:, : tb * HALF],
                    )
                if last:
                    # final subframe j=F: S-half of the last frame,
                    # straight from estsb (no M<128 matmul)
                    nc.sync.dma_start(
                        out=out[c, F * HALF : F * HALF + HALF].rearrange(
                            "(p w) -> p w", p=HALF
                        ),
                        in_=estsb[SOFF : SOFF + HALF, 128:129],
                    )
                # absorb each out-DMA read-completion (WAR) into a DVE
                # write so the next strip's tile writers need no DMA wait
                nc.vector.memset(ctsb[0:1, 0:1], 0.0)
                if last:
                    nc.vector.memset(estsb[SOFF : SOFF + 1, 128:129], 0.0)

            pending = None
            raw = None
            raw_eng = nc.gpsimd if cast_dma else nc.sync
            for c in range(C):
                for s in range(nstrips):
                    f0 = s * STRIP
                    nf = min(STRIP, F - f0)
                    last = s == nstrips - 1
                    assert nf % 128 == 0
                    tb = nf // 128

                    raw = rawp.tile([128, STRIP // 128, E], mdt, tag="raw", name=f"raw_{c}_{s}")
                    with tc.high_priority(offset=90):
                        # p-major: partition p <- frames f0+tb*p..+tb-1, one
                        # contiguous tb*2KB HBM read run per partition
                        raw_eng.dma_start(
                            out=raw[:, :tb, :],
                            in_=mix[c, f0 : f0 + nf, :].rearrange(
                                "(p t) e -> p t e", p=128
                            ),
                        )

                    est = pestp.tile([128, STRIP], F32, tag="est", name=f"est_{c}_{s}")
                    for q in range(NCHUNK):
                        ptr = ptransp.tile(
                            [128, STRIP], mdt, tag="ptr", name=f"ptr_{c}_{s}_{q}"
                        )
                        for t in range(tb):
                            nc.tensor.transpose(
                                ptr[:, t * 128 : (t + 1) * 128],
                                raw[:, t, q * 128 : (q + 1) * 128],
                                id128_sb[:],
                            )
                        mxt = mixtp.tile(
                            [128, STRIP], mdt, tag="mixT", name=f"mxt_{c}_{s}_{q}"
                        )
                        hn = nf // 2
                        nc.vector.tensor_copy(out=mxt[:, :hn], in_=ptr[:, :hn])
                        nc.scalar.copy(out=mxt[:, hn:nf], in_=ptr[:, hn:nf])
                        nc.tensor.matmul(
                            est[:, :nf],
                            lhsT=wt_sb[:, q, :],
                            rhs=mxt[:, :nf],
                            start=(q == 0),
                            stop=(q == NCHUNK - 1),
                        )

                    if pending is not None:
                        emit_tail(*pending)
                    pending = (c, s, f0, nf, last, est)
            emit_tail(*pending)
    _prune_redundant_waits(nc)
    return nc


def _prune_redundant_waits(nc):
    """Drop semaphore waits that are transitively guaranteed.

    Tile's add_semaphores is per-proc minimal but not transitively minimal,
    and several hardware instruction structs (the f32r self-loading
    LDWEIGHTS, HWDGE ring entries) have a single sync-wait slot, so extra
    waits fail walrus codegen ("Too many sync wait commands").

    Soundness: semaphores only increase during execution, and every
    dispatch unit (engine NX, HWDGE ring) executes wait-then-dispatch in
    program order. Hence (a) knowledge carried by the same proc's earlier
    instructions remains true, and (b) a wait (s >= v) is redundant if the
    producer instruction that raised s to v itself had knowledge implying
    it. Additionally, PE-self waits on Matmults are WAW guards for the
    64-deep LDWEIGHTS reorder window; actual MATMULs are strict-FIFO
    (pc-monotone start and end) and LDWEIGHTS only reads SBUF whose
    writers' waits are kept, so they are droppable when another wait
    remains."""
    insts = [i for blk in nc.m.functions[0].blocks for i in blk.instructions]

    # Monotonicity only holds for sems that are never decremented. Engine and
    # DMA sems only see sem-inc / positive sem-add-imm; the barrier_* sems
    # (preamble + kernel tail) use sem-dec/sem-sub and are left untouched.
    unsafe_sems = set()
    for inst in insts:
        si = inst.sync_info
        if si is None:
            continue
        for u in si.on_update or []:
            if u.sync_type != "semaphore":
                continue
            if u.update_mode not in ("sem-inc", "sem-add-imm") or (
                u.update_mode == "sem-add-imm" and u.update_value <= 0
            ):
                unsafe_sems.add(u.id)

    R = {}  # proc -> {sem_id: guaranteed value}
    sem_cum = {}  # sem_id -> cumulative update value
    producer_know = {}  # sem_id -> [(cum_value, knowledge)] in order

    def implied(w, know):
        return know.get(w.id, 0) >= w.wait_value

    def know_of_wait(w):
        k = {w.id: w.wait_value}
        for cv, pk in producer_know.get(w.id, []):
            if cv >= w.wait_value:
                for s2, v2 in pk.items():
                    k[s2] = max(k.get(s2, 0), v2)
                break
        return k

    for inst in insts:
        si = inst.sync_info
        if si is None:
            continue
        waits = list(si.on_wait or [])
        p = str(inst.engine)
        base = dict(R.get(p, {}))
        if any(
            w.sync_type != "semaphore"
            or w.wait_reg is not None
            or w.wait_mode != "sem-ge-imm"
            or w.id in unsafe_sems
            for w in waits
        ):
            kept = waits  # don't touch register/non-sem/barrier waits
        else:
            kept = []
            live = [w for w in waits if not implied(w, base)]
            # prefer a single wait whose producer knowledge implies the rest
            single = None
            for w in live:
                kw = dict(base)
                for s2, v2 in know_of_wait(w).items():
                    kw[s2] = max(kw.get(s2, 0), v2)
                if all(o is w or implied(o, kw) for o in live):
                    single = w
                    break
            if single is not None:
                kept = [single]
            else:
                # greedy: keep a wait only if not implied by base + kept so far
                for w in sorted(live, key=lambda w: -w.wait_value):
                    if not implied(w, base):
                        kept.append(w)
                        for s2, v2 in know_of_wait(w).items():
                            base[s2] = max(base.get(s2, 0), v2)
            if len(kept) > 1:
                # serial in-order engines: own-sem waits are satisfied by
                # the time the instruction executes (PE MATMULs are
                # pc-monotone; DVE/ACT are single-pipeline serial)
                own = {"PE": "PE_", "DVE": "DVE_", "Activation": "Activation_"}.get(
                    str(inst.engine).split(".")[-1]
                )
                if own is not None:
                    nonself = [w for w in kept if not w.ant_name.startswith(own)]
                    if nonself:
                        kept = nonself
            if len(kept) != len(waits):
                si.on_wait = kept
        # final knowledge for this inst (all original waits still held at
        # runtime even if pruned from the emitted instruction)
        know = dict(R.get(p, {}))
        for w in waits:
            if (
                w.sync_type == "semaphore"
                and w.wait_reg is None
                and w.wait_mode == "sem-ge-imm"
                and w.id not in unsafe_sems
            ):
                for s2, v2 in know_of_wait(w).items():
                    know[s2] = max(know.get(s2, 0), v2)
        R[p] = know
        for u in si.on_update or []:
            if u.sync_type != "semaphore" or u.id in unsafe_sems:
                continue
            sem_cum[u.id] = sem_cum.get(u.id, 0) + u.update_value
            producer_know.setdefault(u.id, []).append((sem_cum[u.id], dict(know)))


_NC_CACHE = {}


def _get_nc(**kw):
    key = tuple(sorted(kw.items()))
    if key not in _NC_CACHE:
        _NC_CACHE[key] = build_decoder(**kw)
    return _NC_CACHE[key]


def prep_aux_inputs(basis_weight, mix_dt="bf16"):
    import ml_dtypes

    aux_np = {"bf16": ml_dtypes.bfloat16, "f32r": np.float32, "f32": np.float32}[
        mix_dt
    ]
    wt = np.zeros((E, 128), dtype=np.float32)
    wt[:, 0:HALF] = basis_weight.T[:, 0:HALF]
    wt[:, SOFF : SOFF + HALF] = basis_weight.T[:, HALF:W]
    id128 = np.eye(128, dtype=np.float32)
    sel = np.zeros((128, W), dtype=np.float32)
    for j in range(HALF):
        sel[j, j] = 1.0
        sel[SOFF + j, HALF + j] = 1.0
    return wt.astype(aux_np), id128.astype(aux_np), sel


def kernel(mixture_w, basis_weight, _trace=False, **build_kw):
    mixture_w = np.ascontiguousarray(mixture_w, dtype=np.float32)
    basis_weight = np.ascontiguousarray(basis_weight, dtype=np.float32)
    assert mixture_w.shape == (B, C, F, E), mixture_w.shape
    assert basis_weight.shape == (W, E), basis_weight.shape

    nc = _get_nc(**build_kw)
    wt, id128, sel = prep_aux_inputs(
        basis_weight, mix_dt=build_kw.get("mix_dt", "bf16")
    )
    in_maps = [
        {"mixture_w": mixture_w[i], "wt": wt, "id128": id128, "sel": sel}
        for i in range(N_CORES)
    ]
    res = run_bass_kernel_spmd(
        nc, in_maps, core_ids=list(range(N_CORES)), trace=_trace
    )
    out = np.stack([res.results[i]["out"] for i in range(N_CORES)], axis=0)
    if _trace:
        kernel.last_exec_time_ns = res.exec_time_ns
        kernel.last_result = res
    return out



# revision 10
# speedup vs baseline: 1.0532x; 1.0087x over previous
"""Trainium2 kernel for nn_Decoder_52664888983802.

est = einsum('bckE,wE->bckw', mixture_w, basis_weight); out = overlap_add(est, 8).

Sharding: batch dim (8) -> one batch row per NeuronCore (data parallel, no
collectives). Each core: mix [2, 16000, 512] f32 -> out [2, 128008] f32.
Measured ~250-280 us on silicon (HBM roofline ~185 us; rel err ~2.4e-3).

Per-core pipeline, 512-frame strips, mix path in bf16:
  SWDGE DMA load with f32->bf16 cast, raw [128, tb=4, 512] in P-MAJOR frame
  order: partition p holds frames f0+tb*p .. f0+tb*p+tb-1, i.e. one
  contiguous 8 KB HBM run per partition (128 big descriptors per strip
  instead of 516 2KB ones -- keeps Q7 SWDGE emission off the critical path
  and SDMA packets large).
  -> 16 PE transposes via identity => mixT chunks [128 e, 512 f] in PSUM,
     est column c = t*128+j <-> frame tb*j + t (block t = frames == t mod tb)
  -> PSUM->SBUF copies (split DVE/ACT halves)
  -> 4 accumulating bf16 matmuls, stationary wt [128, 128] (W1 at cols 0-7,
     W2 at cols 32-39, rest zero; 128 cols keeps FWL on) => est [128, nf] PSUM
  -> est PSUM->SBUF reordered as [halo | b_{tb-1} | b_0 .. b_{tb-2}], so the
     overlap-add shift-by-one-frame becomes contiguous 128-col slices:
     F slice of residue r = block r, S slice = block r-1 (r=0: halo+b_{tb-1}).
     estsb col 128 is always the strip's last frame (next strip's halo).
  -> overlap-add folded into the output transpose: per residue r, two
     accumulating K=128 is_transpose matmuls with one-hot column selectors
     (F rows 0-7, S rows 32-39; K<128 matmuls fault at runtime on this
     stack, hence the selector trick) => ct[j, r*8+k] = out subframe
     tb*j+r -> ct [128, tb*8] PSUM -> SBUF -> DMA out with one contiguous
     128B run per partition.
  Final subframe j=16000 is DMA'd straight from est_sb's S rows.
  The output side is traced one strip late (software pipelining), and
  _prune_redundant_waits post-processes Tile's semaphores: several hw
  instruction structs accept a single foreign sync wait, so transitively
  implied waits are dropped (sems are monotonic and dispatch is in-order)
  and serial-engine self-waits are removed when paired with a data wait.
"""

import math
import sys

sys.path.insert(0, "/opt/trn_rl_repo")

import numpy as np

import concourse.bass as bass
import concourse.mybir as mybir
import concourse.tile as tile
from concourse.bass_utils import run_bass_kernel_spmd

F32 = mybir.dt.float32
F32R = mybir.dt.float32r

B, C, F, E, W = 8, 2, 16000, 512, 16
HALF = W // 2
SOFF = 32  # partition offset of the S-half in est
OUTLEN = HALF * (F - 1) + W  # 128008
N_CORES = 8


def build_decoder(C=C, F=F, E=E, W=W, STRIP=512, mix_dt="bf16"):
    HALF = W // 2
    NCHUNK = E // 128
    OUTLEN = HALF * (F - 1) + W

    mdt = {"bf16": mybir.dt.bfloat16, "f32r": F32R, "f32": F32}[mix_dt]
    cast_dma = mix_dt == "bf16"
    nc = bass.Bass()
    mix = nc.declare_dram_parameter(
        "mixture_w", [C, F, E], F32 if cast_dma else mdt, isOutput=False
    )
    wt = nc.declare_dram_parameter("wt", [E, 128], mdt, isOutput=False)
    id128 = nc.declare_dram_parameter("id128", [128, 128], mdt, isOutput=False)
    sel = nc.declare_dram_parameter("sel", [128, W], F32, isOutput=False)
    out = nc.declare_dram_parameter("out", [C, OUTLEN], F32, isOutput=True)

    nstrips = math.ceil(F / STRIP)

    with tile.TileContext(nc) as tc:
        with (
            tc.tile_pool(name="consts", bufs=1) as consts,
            tc.tile_pool(name="rawp", bufs=8) as rawp,
            tc.tile_pool(name="mixtp", bufs=8) as mixtp,
            tc.tile_pool(name="estsbp", bufs=5) as estsbp,
            tc.tile_pool(name="ctsbp", bufs=4) as ctsbp,
            tc.tile_pool(name="ptransp", bufs=3, space="PSUM") as ptransp,
            tc.tile_pool(name="pestp", bufs=3, space="PSUM") as pestp,
            tc.tile_pool(name="pctp", bufs=2, space="PSUM") as pctp,
        ):
            id128_sb = consts.tile([128, 128], mdt)
            nc.sync.dma_start(out=id128_sb[:], in_=id128[:])
            # selector: cols 0-7 pick est rows 0-7 (F), cols 8-15 pick rows
            # 32-39 (S) -- K=128 transposes only (K<128 faults at runtime)
            sel_sb = consts.tile([128, W], F32)
            nc.sync.dma_start(out=sel_sb[:], in_=sel[:])
            wt_sb = consts.tile([128, NCHUNK, 128], mdt)
            nc.sync.dma_start(out=wt_sb[:], in_=wt.rearrange("(q p) w -> p q w", p=128))

            # Warm-up PE ops: consume each const right after its DMA so that
            # steady-state PE instructions never need more than one
            # cross-engine wait (the f32r self-loading LDWEIGHTS struct has a
            # single sync-wait slot).
            warm = ptransp.tile([128, 128], mdt, tag="ptr", name="warm_t")
            nc.tensor.transpose(warm[:], id128_sb[:], id128_sb[:])
            warm2 = pctp.tile([W, W], F32, tag="ct", name="warm_ct")
            nc.tensor.matmul(
                warm2[:], lhsT=sel_sb[:], rhs=sel_sb[:], is_transpose=True
            )
            warm3 = pestp.tile([128, HALF], F32, tag="est", name="warm_mm")
            nc.tensor.matmul(
                warm3[:], lhsT=wt_sb[:, 0, :], rhs=wt_sb[:, 0, :HALF]
            )


            def pe_pad(n=1):
                # Keep the PE HAM clock-gate warm: standalone LDWEIGHTS
                # (no PSUM write, no cross-engine deps). Placed between
                # LDW/MM pairs only -- never between a matmul's own
                # LDWEIGHTS and its MATMUL (program order on PE).
                for _ in range(n):
                    nc.tensor.ldweights(weights=id128_sb[:, :128])

            prev_estsb = None

            def emit_tail(c, s, f0, nf, last, est):
                # Deferred output side of a strip: traced one strip late so
                # the scheduler interleaves the next strip's PE work with
                # these DVE/ACT copies (software pipelining).
                nonlocal prev_estsb
                tb = nf // 128
                # est_sb col 0 is the halo (previous strip's last frame);
                # block order [halo | b_{tb-1} | b_0 .. b_{tb-2}] so every
                # F/S slice below is 128 contiguous cols.
                estsb = estsbp.tile(
                    [128, STRIP + 2], F32, tag="estsb", name=f"estsb_{c}_{s}"
                )
                # b_{tb-1} first (cols 1..129) -- estsb col 128 = last frame
                nc.vector.tensor_copy(
                    out=estsb[:, 1:129], in_=est[:, (tb - 1) * 128 : nf]
                )
                if tb > 1:
                    # remaining blocks b_0..b_{tb-2}, split DVE/ACT
                    hn = ((tb - 1) // 2) * 128
                    if hn:
                        nc.vector.tensor_copy(
                            out=estsb[:, 129 : 129 + hn], in_=est[:, 0:hn]
                        )
                    nc.scalar.copy(
                        out=estsb[:, 129 + hn : 129 + (tb - 1) * 128],
                        in_=est[:, hn : (tb - 1) * 128],
                    )
                if s == 0:
                    nc.vector.memset(estsb[:, 0:1], 0.0)
                else:
                    nc.vector.tensor_copy(
                        out=estsb[:, 0:1], in_=prev_estsb[:, 128:129]
                    )
                prev_estsb = estsb

                # output transpose with overlap-add folded in (K=128,
                # selector picks F rows 0-7 / S rows 32-39).
                # ct[j, r*8+k] = F[tb*j+r][k] + S[tb*j+r-1][k]
                def fstart(r):
                    return 1 if r == tb - 1 else 129 + 128 * r

                ct = pctp.tile([128, tb * HALF], F32, tag="ct", name=f"ct_{c}_{s}")
                for r in range(tb):
                    fs = fstart(r)
                    ss = 0 if r == 0 else fstart(r - 1)
                    nc.tensor.matmul(
                        ct[:, r * HALF : (r + 1) * HALF],
                        lhsT=estsb[:, fs : fs + 128], rhs=sel_sb[:, 0:HALF],
                        is_transpose=True, start=True, stop=False,
                    )
                    nc.tensor.matmul(
                        ct[:, r * HALF : (r + 1) * HALF],
                        lhsT=estsb[:, ss : ss + 128], rhs=sel_sb[:, HALF:W],
                        is_transpose=True, start=False, stop=True,
                    )
                    pe_pad(1)

                ctsb = ctsbp.tile(
                    [128, tb * HALF], F32, tag="ctsb", name=f"ctsb_{c}_{s}"
                )
                nc.vector.tensor_copy(out=ctsb[:, : tb * HALF], in_=ct[:])
                with tc.high_priority(offset=-150):
                    # partition j covers out subframes tb*j+0 .. tb*j+tb-1:
                    # one contiguous tb*32-byte run per partition
                    nc.sync.dma_start(
                        out=out[
                            c, f0 * HALF : (f0 + nf) * HALF
                        ].rearrange("(p x) -> p x", p=128),
                        in_=ctsb[:, : tb * HALF],
                    )
                if last:
                    # final subframe j=F: S-half of the last frame,
                    # straight from estsb (no M<128 matmul)
                    nc.sync.dma_start(
                        out=out[c, F * HALF : F * HALF + HALF].rearrange(
                            "(p w) -> p w", p=HALF
                        ),
                        in_=estsb[SOFF : SOFF + HALF, 128:129],
                    )
                # absorb each out-DMA read-completion (WAR) into a DVE
                # write so the next strip's tile writers need no DMA wait
                nc.vector.memset(ctsb[0:1, 0:1], 0.0)
                if last:
                    nc.vector.memset(estsb[SOFF : SOFF + 1, 128:129], 0.0)

            pending = None
            raw = None
            raw_eng = nc.gpsimd if cast_dma else nc.sync
            for c in range(C):
                for s in range(nstrips):
                    f0 = s * STRIP
                    nf = min(STRIP, F - f0)
                    last = s == nstrips - 1
                    assert nf % 128 == 0
                    tb = nf // 128

                    raw = rawp.tile([128, STRIP // 128, E], mdt, tag="raw", name=f"raw_{c}_{s}")
                    with tc.high_priority(offset=90):
                        # p-major: partition p <- frames f0+tb*p..+tb-1, one
                        # contiguous tb*2KB HBM read run per partition
                        raw_eng.dma_start(
                            out=raw[:, :tb, :],
                            in_=mix[c, f0 : f0 + nf, :].rearrange(
                                "(p t) e -> p t e", p=128
                            ),
                        )

                    est = pestp.tile([128, STRIP], F32, tag="est", name=f"est_{c}_{s}")
                    for q in range(NCHUNK):
                        ptr = ptransp.tile(
                            [128, STRIP], mdt, tag="ptr", name=f"ptr_{c}_{s}_{q}"
                        )
                        for t in range(tb):
                            nc.tensor.transpose(
                                ptr[:, t * 128 : (t + 1) * 128],
                                raw[:, t, q * 128 : (q + 1) * 128],
                                id128_sb[:],
                            )
                        mxt = mixtp.tile(
                            [128, STRIP], mdt, tag="mixT", name=f"mxt_{c}_{s}_{q}"
                        )
                        hn = nf // 2
                        nc.vector.tensor_copy(out=mxt[:, :hn], in_=ptr[:, :hn])
                        nc.scalar.copy(out=mxt[:, hn:nf], in_=ptr[:, hn:nf])
                        nc.tensor.matmul(
                            est[:, :nf],
                            lhsT=wt_sb[:, q, :],
                            rhs=mxt[:, :nf],
                            start=(q == 0),
                            stop=(q == NCHUNK - 1),
                        )
                        pe_pad(1)

                    if pending is not None:
                        emit_tail(*pending)
                    pending = (c, s, f0, nf, last, est)
            emit_tail(*pending)
    _prune_redundant_waits(nc)
    return nc


def _prune_redundant_waits(nc):
    """Drop semaphore waits that are transitively guaranteed.

    Tile's add_semaphores is per-proc minimal but not transitively minimal,
    and several hardware instruction structs (the f32r self-loading
    LDWEIGHTS, HWDGE ring entries) have a single sync-wait slot, so extra
    waits fail walrus codegen ("Too many sync wait commands").

    Soundness: semaphores only increase during execution, and every
    dispatch unit (engine NX, HWDGE ring) executes wait-then-dispatch in
    program order. Hence (a) knowledge carried by the same proc's earlier
    instructions remains true, and (b) a wait (s >= v) is redundant if the
    producer instruction that raised s to v itself had knowledge implying
    it. Additionally, PE-self waits on Matmults are WAW guards for the
    64-deep LDWEIGHTS reorder window; actual MATMULs are strict-FIFO
    (pc-monotone start and end) and LDWEIGHTS only reads SBUF whose
    writers' waits are kept, so they are droppable when another wait
    remains."""
    insts = [i for blk in nc.m.functions[0].blocks for i in blk.instructions]

    # Monotonicity only holds for sems that are never decremented. Engine and
    # DMA sems only see sem-inc / positive sem-add-imm; the barrier_* sems
    # (preamble + kernel tail) use sem-dec/sem-sub and are left untouched.
    unsafe_sems = set()
    for inst in insts:
        si = inst.sync_info
        if si is None:
            continue
        for u in si.on_update or []:
            if u.sync_type != "semaphore":
                continue
            if u.update_mode not in ("sem-inc", "sem-add-imm") or (
                u.update_mode == "sem-add-imm" and u.update_value <= 0
            ):
                unsafe_sems.add(u.id)

    R = {}  # proc -> {sem_id: guaranteed value}
    sem_cum = {}  # sem_id -> cumulative update value
    producer_know = {}  # sem_id -> [(cum_value, knowledge)] in order

    def implied(w, know):
        return know.get(w.id, 0) >= w.wait_value

    def know_of_wait(w):
        k = {w.id: w.wait_value}
        for cv, pk in producer_know.get(w.id, []):
            if cv >= w.wait_value:
                for s2, v2 in pk.items():
                    k[s2] = max(k.get(s2, 0), v2)
                break
        return k

    for inst in insts:
        si = inst.sync_info
        if si is None:
            continue
        waits = list(si.on_wait or [])
        p = str(inst.engine)
        base = dict(R.get(p, {}))
        if any(
            w.sync_type != "semaphore"
            or w.wait_reg is not None
            or w.wait_mode != "sem-ge-imm"
            or w.id in unsafe_sems
            for w in waits
        ):
            kept = waits  # don't touch register/non-sem/barrier waits
        else:
            kept = []
            live = [w for w in waits if not implied(w, base)]
            # prefer a single wait whose producer knowledge implies the rest
            single = None
            for w in live:
                kw = dict(base)
                for s2, v2 in know_of_wait(w).items():
                    kw[s2] = max(kw.get(s2, 0), v2)
                if all(o is w or implied(o, kw) for o in live):
                    single = w
                    break
            if single is not None:
                kept = [single]
            else:
                # greedy: keep a wait only if not implied by base + kept so far
                for w in sorted(live, key=lambda w: -w.wait_value):
                    if not implied(w, base):
                        kept.append(w)
                        for s2, v2 in know_of_wait(w).items():
                            base[s2] = max(base.get(s2, 0), v2)
            if len(kept) > 1:
                # serial in-order engines: own-sem waits are satisfied by
                # the time the instruction executes (PE MATMULs are
                # pc-monotone; DVE/ACT are single-pipeline serial)
                own = {"PE": "PE_", "DVE": "DVE_", "Activation": "Activation_"}.get(
                    str(inst.engine).split(".")[-1]
                )
                if own is not None:
                    nonself = [w for w in kept if not w.ant_name.startswith(own)]
                    if nonself:
                        kept = nonself
            if len(kept) != len(waits):
                si.on_wait = kept
        # final knowledge for this inst (all original waits still held at
        # runtime even if pruned from the emitted instruction)
        know = dict(R.get(p, {}))
        for w in waits:
            if (
                w.sync_type == "semaphore"
                and w.wait_reg is None
                and w.wait_mode == "sem-ge-imm"
                and w.id not in unsafe_sems
            ):
                for s2, v2 in know_of_wait(w).items():
                    know[s2] = max(know.get(s2, 0), v2)
        R[p] = know
        for u in si.on_update or []:
            if u.sync_type != "semaphore" or u.id in unsafe_sems:
                continue
            sem_cum[u.id] = sem_cum.get(u.id, 0) + u.update_value
            producer_know.setdefault(u.id, []).append((sem_cum[u.id], dict(know)))


_NC_CACHE = {}


def _get_nc(**kw):
    key = tuple(sorted(kw.items()))
    if key not in _NC_CACHE:
        _NC_CACHE[key] = build_decoder(**kw)
    return _NC_CACHE[key]


def prep_aux_inputs(basis_weight, mix_dt="bf16"):
    import ml_dtypes

    aux_np = {"bf16": ml_dtypes.bfloat16, "f32r": np.float32, "f32": np.float32}[
        mix_dt
    ]
    wt = np.zeros((E, 128), dtype=np.float32)
    wt[:, 0:HALF] = basis_weight.T[:, 0:HALF]
    wt[:, SOFF : SOFF + HALF] = basis_weight.T[:, HALF:W]
    id128 = np.eye(128, dtype=np.float32)
    sel = np.zeros((128, W), dtype=np.float32)
    for j in range(HALF):
        sel[j, j] = 1.0
        sel[SOFF + j, HALF + j] = 1.0
    return wt.astype(aux_np), id128.astype(aux_np), sel


def kernel(mixture_w, basis_weight, _trace=False, **build_kw):
    mixture_w = np.ascontiguousarray(mixture_w, dtype=np.float32)
    basis_weight = np.ascontiguousarray(basis_weight, dtype=np.float32)
    assert mixture_w.shape == (B, C, F, E), mixture_w.shape
    assert basis_weight.shape == (W, E), basis_weight.shape

    nc = _get_nc(**build_kw)
    wt, id128, sel = prep_aux_inputs(
        basis_weight, mix_dt=build_kw.get("mix_dt", "bf16")
    )
    in_maps = [
        {"mixture_w": mixture_w[i], "wt": wt, "id128": id128, "sel": sel}
        for i in range(N_CORES)
    ]
    res = run_bass_kernel_spmd(
        nc, in_maps, core_ids=list(range(N_CORES)), trace=_trace
    )
    out = np.stack([res.results[i]["out"] for i in range(N_CORES)], axis=0)
    if _trace:
        kernel.last_exec_time_ns = res.exec_time_ns
        kernel.last_result = res
    return out

